# revision 2
# baseline (speedup 1.0000x reference)
"""nn_BinaryConv2D Trainium2 kernel.

out = conv2d(sign(x), sign(w)), 3x3, stride 1, SAME, NHWC/HWIO.
x [64, 128, 128, 64] fp32, w [3, 3, 64, 64] fp32 -> out [64, 128, 128, 64] fp32.

Sharding: data-parallel over batch across 8 NeuronCores (8 images/core);
the tiny weight is sign()ed host-side, packed into per-tap stacks, and
replicated to every core.

Per-core scheme (all shapes hardcoded):
- Input rows are loaded 8 at a time as [128, 512] fp32 SBUF tiles whose
  partition is the pixel-pair index within each 2-row subblock (512B
  contiguous DRAM per partition chunk).
- ACT Sign converts to +-1 bf16.
- DMA xbar transpose turns each 2-row chunk [128 pairs, 128=(2px,ci)] into a
  polyphase stack [128=(even-pixel ci | odd-pixel ci), 128 pairs].
- DVE places phase stacks into a per-image "mega" tile with 66-col slots
  (64 pair columns + shared zero-pad columns) and zero slots for the SAME
  padding rows -1/128.
- Conv = 12 matmuls per PSUM group of g row-slots (N=g*66): for each dy one
  K=128 matmul (two dx taps via the two pixel phases) plus one K=64 matmul
  (third dx tap, shifted one pair column) per output-parity strip; strips
  are packed into PSUM partitions 0:64 / 64:128 (PE column tiling).
- DVE strips pad columns and casts PSUM fp32 -> fp16 (exact: outputs are
  integers in [-576, 576]).
- DMA xbar transpose converts channel-major [co-stacks, pairs] to
  pixel-major [pairs, 2px*co] tiles stored straight into NHWC DRAM (fp16,
  upcast to fp32 on the host — exact).
"""

from contextlib import ExitStack

import numpy as np
import ml_dtypes

import concourse.bass as bass
import concourse.tile as tile
from concourse import mybir
from concourse.vector_clock import ScopedClock, VectorClock

H = W = 128
C = 64
SW = 66
OFF = 1
MEGA_COLS = OFF + (H + 2) * SW + 1  # 8582
N_CORES = 8
NIMG = 8  # images per core


# ---------------------------------------------------------------------------
# Workaround for this container's walrus: CTRL instructions support only ONE
# sync-wait slot, but Tile's tail drain attaches one wait per live proc.
# Split the waits across single-wait NoOps on the SP engine (in-order), then
# drain waitless.
def _drain_and_barrier_split(self, tick_clock, wait_clock):
    nc = self.nc
    vc = tick_clock.global_clock
    n = len(vc)
    for i in range(n):
        if vc[i] > 0:
            sub = VectorClock([0] * n)
            sub.require_at_least(i, vc[i])
            nop = nc.sync.nop(nofuse=True)
            wait_clock.add_sem_waits(nop.ins, ScopedClock({None: sub}))
    nc.sync.drain()
    nc.all_engine_barrier()
    assert self.sems is not None
    popped = nc._tile_sem_poison_stack.pop()
    assert popped is self._sem_poison
    nc.clear_and_free_semaphores(list(self.sems.allocated().values()))
    nc.all_engine_barrier()


tile.TileContext._drain_and_barrier = _drain_and_barrier_split


# The same walrus limit applies to every instruction: at most one sync wait.
# Tile freely emits multi-wait instructions, so rewrite the BIR JSON right
# before compilation: hoist all but the last wait of each instruction onto
# fresh same-engine NoOps inserted immediately before it (engines execute
# their instruction stream in order, so the waits still gate the original
# instruction).
def _split_multi_waits_json(bir_bytes):
    import json as _json

    bir = _json.loads(bir_bytes)
    n = 0
    for fn in bir.get("functions", []):
        for blk in fn.get("blocks", []):
            insts = blk.get("instructions", [])
            out = []
            for ins in insts:
                si = ins.get("sync_info")
                if si:
                    waits = si.get("on_wait") or []
                    if len(waits) > 1:
                        for wv in waits[:-1]:
                            n += 1
                            out.append(
                                {
                                    "debug": ins.get("debug", 0),
                                    "engine": ins["engine"],
                                    "ins": [],
                                    "outs": [],
                                    "name": f"I-wsplit-{n}",
                                    "opcode": "NoOp",
                                    "sync_info": {
                                        "on_update": [],
                                        "on_wait": [wv],
                                    },
                                }
                            )
                        si["on_wait"] = [waits[-1]]
                out.append(ins)
            blk["instructions"] = out
    return _json.dumps(bir).encode()


def _install_compile_hook():
    from concourse import bass_utils as _bu
    from concourse import bass2jax as _b2j

    if getattr(_bu, "_orig_compile_bir_kernel", None) is None:
        _bu._orig_compile_bir_kernel = _bu.compile_bir_kernel

        def _patched(bir_json, tmpdir, neff_name="file.neff"):
            return _bu._orig_compile_bir_kernel(
                _split_multi_waits_json(bir_json), tmpdir, neff_name=neff_name
            )

        _bu.compile_bir_kernel = _patched
        _b2j.compile_bir_kernel = _patched


_install_compile_hook()
# ---------------------------------------------------------------------------


def build_nc(nimg=NIMG, gsize=6):
    nc = bass.Bass()
    x = nc.dram_tensor("x", [nimg, H, W, C], mybir.dt.float32, kind="ExternalInput")
    wt = nc.dram_tensor("wt", [128, 12 * C], mybir.dt.bfloat16, kind="ExternalInput")
    y = nc.dram_tensor("y", [nimg, H, W, C], mybir.dt.float16, kind="ExternalOutput")

    with tile.TileContext(nc) as tc, ExitStack() as ctx:
        wpool = ctx.enter_context(tc.tile_pool(name="wpool", bufs=1))
        mega_pool = ctx.enter_context(tc.tile_pool(name="mega", bufs=2))
        in_pool = ctx.enter_context(tc.tile_pool(name="inp", bufs=3))
        sg_pool = ctx.enter_context(tc.tile_pool(name="sg", bufs=3))
        tmp_pool = ctx.enter_context(tc.tile_pool(name="tmp", bufs=3))
        psum_pool = ctx.enter_context(tc.tile_pool(name="ps", bufs=4, space="PSUM"))
        cm_pool = ctx.enter_context(tc.tile_pool(name="cm", bufs=3))
        ot_pool = ctx.enter_context(tc.tile_pool(name="ot", bufs=6))

        wt_sb = wpool.tile([128, 12 * C], mybir.dt.bfloat16)
        nc.sync.dma_start(out=wt_sb[:], in_=wt[:])

        groups = []
        r0 = 0
        while r0 < H:
            g = min(gsize, H - r0)
            groups.append((r0, g))
            r0 += g

        for img in range(nimg):
            mega = mega_pool.tile([128, MEGA_COLS], mybir.dt.bfloat16)
            # zero-pad columns: {66m+1} (left pads) and {66m+66} (right pads)
            # covered as pairs {66m+66, 66m+67} plus edge cols.
            nc.gpsimd.memset(mega[:, 0:2], 0.0)
            pads = mega[:, SW : SW + 129 * SW].rearrange(
                "p (s c) -> p s c", c=SW
            )[:, :, 0:2]
            nc.gpsimd.memset(pads, 0.0)
            nc.gpsimd.memset(mega[:, MEGA_COLS - 2 : MEGA_COLS], 0.0)
            # zero slots for SAME-pad rows -1 and H
            nc.gpsimd.memset(mega[:, 2 : 2 + 64], 0.0)
            b128 = OFF + (H + 1) * SW + 1
            nc.gpsimd.memset(mega[:, b128 : b128 + 64], 0.0)

            # input: 16 chunks of 8 rows
            for t in range(H // 8):
                xt = in_pool.tile([128, 512], mybir.dt.float32)
                src = x[img, 8 * t : 8 * t + 8]  # [8, 128, 64]
                src_v = src.rearrange(
                    "(u r) (j two) c -> r j u two c", r=2, two=2
                ).rearrange("r j u two c -> (r j) u (two c)")
                nc.sync.dma_start(
                    out=xt[:].rearrange("p (u f) -> p u f", f=128), in_=src_v
                )

                sg = sg_pool.tile([128, 512], mybir.dt.bfloat16)
                nc.scalar.activation(
                    sg[:], xt[:], mybir.ActivationFunctionType.Sign
                )

                tmp = tmp_pool.tile([128, 512], mybir.dt.bfloat16)
                for u in range(4):
                    nc.sync.dma_start_transpose(
                        out=tmp[:, 128 * u : 128 * (u + 1)],
                        in_=sg[:, 128 * u : 128 * (u + 1)],
                    )

                b0 = OFF + (8 * t + 1) * SW + 1
                dst = mega[:, b0 : b0 + 8 * SW].rearrange(
                    "p (s c) -> p s c", c=SW
                )[:, :, 0:64]
                nc.vector.tensor_copy(
                    out=dst, in_=tmp[:].rearrange("p (s c) -> p s c", c=64)
                )

            # matmul groups
            for r0, g in groups:
                N = g * SW
                ps = psum_pool.tile(
                    [128, N], mybir.dt.float32, padded_shape=[128, 512]
                )
                # upper strip (even output pixels); its accumulation group
                # completes before the lower strip starts (sim allows one
                # pending group per bank).
                for dy in range(3):
                    ib = OFF + (r0 + dy) * SW
                    nc.tensor.matmul(
                        ps[0:64, :],
                        wt_sb[:, dy * 64 : dy * 64 + 64],
                        mega[:, ib : ib + N],
                        start=(dy == 0),
                        stop=False,
                        tile_position=(0, 0),
                    )
                    nc.tensor.matmul(
                        ps[0:64, :],
                        wt_sb[64:128, (6 + dy) * 64 : (6 + dy) * 64 + 64],
                        mega[64:128, ib - 1 : ib - 1 + N],
                        start=False,
                        stop=(dy == 2),
                        tile_position=(64, 0),
                    )
                # lower strip (odd output pixels)
                for dy in range(3):
                    ib = OFF + (r0 + dy) * SW
                    nc.tensor.matmul(
                        ps[64:128, :],
                        wt_sb[:, (3 + dy) * 64 : (3 + dy) * 64 + 64],
                        mega[:, ib : ib + N],
                        start=(dy == 0),
                        stop=False,
                        tile_position=(0, 64),
                    )
                    nc.tensor.matmul(
                        ps[64:128, :],
                        wt_sb[0:64, (9 + dy) * 64 : (9 + dy) * 64 + 64],
                        mega[0:64, ib + 1 : ib + 1 + N],
                        start=False,
                        stop=(dy == 2),
                        tile_position=(0, 64),
                    )

                cm = cm_pool.tile([128, g * 64], mybir.dt.float16)
                ps_v = ps[:].rearrange("p (s c) -> p s c", c=SW)[:, :, 1:65]
                nc.vector.tensor_copy(
                    out=cm[:].rearrange("p (s c) -> p s c", c=64), in_=ps_v
                )

                for c2 in range(g // 2):
                    ot = ot_pool.tile([128, 128], mybir.dt.float16)
                    nc.sync.dma_start_transpose(
                        out=ot[:], in_=cm[:, 128 * c2 : 128 * c2 + 128]
                    )
                    rr = r0 + 2 * c2
                    ydst = y[img, rr : rr + 2].rearrange(
                        "r (p two) c -> (r p) (two c)", two=2
                    )
                    nc.sync.dma_start(out=ydst, in_=ot[:])

    return nc


def make_wt(w_np):
    """Host-side weight prep: w [3,3,64,64] fp32 -> wt [128, 768] bf16."""
    ws = np.sign(w_np).astype(np.float32)
    wt = np.zeros((128, 12 * C), np.float32)
    for dy in range(3):
        # A: even strip K=128 (rows 0:64 <- w[dy,1], rows 64:128 <- w[dy,2])
        wt[0:64, dy * 64 : dy * 64 + 64] = ws[dy, 1]
        wt[64:128, dy * 64 : dy * 64 + 64] = ws[dy, 2]
        # C: odd strip K=128 (w[dy,0]; w[dy,1])
        wt[0:64, (3 + dy) * 64 : (3 + dy) * 64 + 64] = ws[dy, 0]
        wt[64:128, (3 + dy) * 64 : (3 + dy) * 64 + 64] = ws[dy, 1]
        # B: even strip K=64 on odd partitions (w[dy,0])
        wt[64:128, (6 + dy) * 64 : (6 + dy) * 64 + 64] = ws[dy, 0]
        # D: odd strip K=64 on even partitions (w[dy,2])
        wt[0:64, (9 + dy) * 64 : (9 + dy) * 64 + 64] = ws[dy, 2]
    return wt.astype(ml_dtypes.bfloat16)


_NC_CACHE = {}


def get_nc():
    if "nc" not in _NC_CACHE:
        _NC_CACHE["nc"] = build_nc()
    return _NC_CACHE["nc"]


def kernel(x, w):
    from concourse.bass_utils import run_bass_kernel_spmd

    x = np.asarray(x, dtype=np.float32)
    w = np.asarray(w, dtype=np.float32)
    assert x.shape == (N_CORES * NIMG, H, W, C) and w.shape == (3, 3, C, C)

    wt = make_wt(w)
    nc = get_nc()
    in_maps = [
        {"x": np.ascontiguousarray(x[c * NIMG : (c + 1) * NIMG]), "wt": wt}
        for c in range(N_CORES)
    ]
    res = run_bass_kernel_spmd(nc, in_maps, list(range(N_CORES)))
    out = np.concatenate(
        [np.asarray(res.results[c]["y"]) for c in range(N_CORES)], axis=0
    )
    return out.astype(np.float32)


# revision 9
# speedup vs baseline: 3.7700x; 3.7700x over previous
"""nn_BinaryConv2D Trainium2 kernel.

out = conv2d(sign(x), sign(w)), 3x3, stride 1, SAME, NHWC/HWIO.
x [64, 128, 128, 64] fp32, w [3, 3, 64, 64] fp32 -> out [64, 128, 128, 64] fp32.

Sharding: data-parallel over batch across 8 NeuronCores (8 images/core);
the tiny weight is sign()ed host-side, packed into per-tap stacks, and
replicated to every core.

Per-core scheme (all shapes hardcoded):
- Input rows are loaded 8 at a time as [128, 512] fp32 SBUF tiles whose
  partition is the pixel-pair index within each 2-row subblock (512B
  contiguous DRAM per partition chunk).
- ACT Sign converts to +-1 bf16.
- DMA xbar transpose turns each 2-row chunk [128 pairs, 128=(2px,ci)] into a
  polyphase stack [128=(even-pixel ci | odd-pixel ci), 128 pairs].
- DVE places phase stacks into a per-image "mega" tile with 66-col slots
  (64 pair columns + shared zero-pad columns) and zero slots for the SAME
  padding rows -1/128.
- Conv = 12 matmuls per PSUM group of g row-slots (N=g*66): for each dy one
  K=128 matmul (two dx taps via the two pixel phases) plus one K=64 matmul
  (third dx tap, shifted one pair column) per output-parity strip; strips
  are packed into PSUM partitions 0:64 / 64:128 (PE column tiling).
- DVE strips pad columns and casts PSUM fp32 -> fp16 (exact: outputs are
  integers in [-576, 576]).
- DMA xbar transpose converts channel-major [co-stacks, pairs] to
  pixel-major [pairs, 2px*co] tiles stored straight into NHWC DRAM (fp16,
  upcast to fp32 on the host — exact).
"""

from contextlib import ExitStack

import numpy as np
import ml_dtypes

import concourse.bass as bass
import concourse.tile as tile
from concourse import mybir
from concourse.vector_clock import ScopedClock, VectorClock
from concourse.tile_rust import add_dep_helper

H = W = 128
C = 64
SW = 66
OFF = 1
MEGA_COLS = OFF + (H + 2) * SW + 1  # 8582
N_CORES = 8
NIMG = 8  # images per core


# ---------------------------------------------------------------------------
# Workaround for this container's walrus: CTRL instructions support only ONE
# sync-wait slot, but Tile's tail drain attaches one wait per live proc.
# Split the waits across single-wait NoOps on the SP engine (in-order), then
# drain waitless.
def _drain_and_barrier_split(self, tick_clock, wait_clock):
    nc = self.nc
    vc = tick_clock.global_clock
    n = len(vc)
    for i in range(n):
        if vc[i] > 0:
            sub = VectorClock([0] * n)
            sub.require_at_least(i, vc[i])
            nop = nc.sync.nop(nofuse=True)
            wait_clock.add_sem_waits(nop.ins, ScopedClock({None: sub}))
    nc.sync.drain()
    nc.all_engine_barrier()
    assert self.sems is not None
    popped = nc._tile_sem_poison_stack.pop()
    assert popped is self._sem_poison
    nc.clear_and_free_semaphores(list(self.sems.allocated().values()))
    nc.all_engine_barrier()


tile.TileContext._drain_and_barrier = _drain_and_barrier_split


# The same walrus limit applies to every instruction: at most one sync wait.
# Tile freely emits multi-wait instructions, so rewrite the BIR JSON right
# before compilation: hoist all but the last wait of each instruction onto
# fresh same-engine NoOps inserted immediately before it (engines execute
# their instruction stream in order, so the waits still gate the original
# instruction).
def _split_multi_waits_json(bir_bytes):
    import json as _json

    bir = _json.loads(bir_bytes)
    n = 0
    for fn in bir.get("functions", []):
        for blk in fn.get("blocks", []):
            insts = blk.get("instructions", [])
            out = []
            for ins in insts:
                si = ins.get("sync_info")
                if si:
                    waits = si.get("on_wait") or []
                    if len(waits) > 1:
                        for wv in waits[:-1]:
                            n += 1
                            out.append(
                                {
                                    "debug": ins.get("debug", 0),
                                    "engine": ins["engine"],
                                    "ins": [],
                                    "outs": [],
                                    "name": f"I-wsplit-{n}",
                                    "opcode": "NoOp",
                                    "sync_info": {
                                        "on_update": [],
                                        "on_wait": [wv],
                                    },
                                }
                            )
                        si["on_wait"] = [waits[-1]]
                out.append(ins)
            blk["instructions"] = out
    return _json.dumps(bir).encode()


def _install_compile_hook():
    from concourse import bass_utils as _bu
    from concourse import bass2jax as _b2j

    if getattr(_bu, "_orig_compile_bir_kernel", None) is None:
        _bu._orig_compile_bir_kernel = _bu.compile_bir_kernel

        def _patched(bir_json, tmpdir, neff_name="file.neff"):
            return _bu._orig_compile_bir_kernel(
                _split_multi_waits_json(bir_json), tmpdir, neff_name=neff_name
            )

        _bu.compile_bir_kernel = _patched
        _b2j.compile_bir_kernel = _patched


_install_compile_hook()
# ---------------------------------------------------------------------------


def build_nc(nimg=NIMG, gsize=7):
    nc = bass.Bass()
    x = nc.dram_tensor("x", [nimg, H, W, C], mybir.dt.float32, kind="ExternalInput")
    wt = nc.dram_tensor("wt", [128, 12 * C], mybir.dt.bfloat16, kind="ExternalInput")
    y = nc.dram_tensor("y", [nimg, H, W, C], mybir.dt.float16, kind="ExternalOutput")

    with tile.TileContext(nc) as tc, ExitStack() as ctx:
        wpool = ctx.enter_context(tc.tile_pool(name="wpool", bufs=1))
        mega_pool = ctx.enter_context(tc.tile_pool(name="mega", bufs=2))
        in_pool = ctx.enter_context(tc.tile_pool(name="inp", bufs=3))
        sg_pool = ctx.enter_context(tc.tile_pool(name="sg", bufs=3))
        tmp_pool = ctx.enter_context(tc.tile_pool(name="tmp", bufs=3))
        psum_pool = ctx.enter_context(tc.tile_pool(name="ps", bufs=4, space="PSUM"))
        cm_pool = ctx.enter_context(tc.tile_pool(name="cm", bufs=3))
        ot_pool = ctx.enter_context(tc.tile_pool(name="ot", bufs=6))

        wt_sb = wpool.tile([128, 12 * C], mybir.dt.bfloat16)
        nc.sync.dma_start(out=wt_sb[:], in_=wt[:])

        groups = []
        r0 = 0
        while r0 < H:
            g = min(gsize, H - r0)
            groups.append((r0, g))
            r0 += g

        for img in range(nimg):
            mega = mega_pool.tile([128, MEGA_COLS], mybir.dt.bfloat16)
            # zero-pad columns: {66m+1} (left pads) and {66m+66} (right pads)
            # covered as pairs {66m+66, 66m+67} plus edge cols.
            nc.gpsimd.memset(mega[:, 0:2], 0.0)
            pads = mega[:, SW : SW + 129 * SW].rearrange(
                "p (s c) -> p s c", c=SW
            )[:, :, 0:2]
            nc.gpsimd.memset(pads, 0.0)
            nc.gpsimd.memset(mega[:, MEGA_COLS - 2 : MEGA_COLS], 0.0)
            # zero slots for SAME-pad rows -1 and H
            nc.gpsimd.memset(mega[:, 2 : 2 + 64], 0.0)
            b128 = OFF + (H + 1) * SW + 1
            nc.gpsimd.memset(mega[:, b128 : b128 + 64], 0.0)

            # input: 4 chunks of 32 rows
            RC = 32  # rows per chunk
            FC = RC * 64  # free elems per chunk (= 16 blocks of 128)
            for t in range(H // RC):
                xt = in_pool.tile([128, FC], mybir.dt.float32)
                src = x[img, RC * t : RC * t + RC]  # [RC, 128, 64]
                src_v = src.rearrange(
                    "(u r) (j two) c -> r j u two c", r=2, two=2
                ).rearrange("r j u two c -> (r j) u (two c)")
                nc.sync.dma_start(
                    out=xt[:].rearrange("p (u f) -> p u f", f=128), in_=src_v
                )

                sg = sg_pool.tile([128, FC], mybir.dt.bfloat16)
                nc.scalar.activation(
                    sg[:], xt[:], mybir.ActivationFunctionType.Sign
                )

                # one batched xbar: G independent 128x128 transposes
                tmp = tmp_pool.tile([128, FC], mybir.dt.bfloat16)
                nc.sync.dma_start_transpose(
                    out=tmp[:].rearrange("p (g f) -> p g f", f=128),
                    in_=sg[:],
                )

                b0 = OFF + (RC * t + 1) * SW + 1
                dst = mega[:, b0 : b0 + RC * SW].rearrange(
                    "p (s c) -> p s c", c=SW
                )[:, :, 0:64]
                nc.vector.tensor_copy(
                    out=dst, in_=tmp[:].rearrange("p (s c) -> p s c", c=64)
                )

            # matmul groups, outputs batched across GB groups per cm tile
            GB = 4
            batches = [groups[i : i + GB] for i in range(0, len(groups), GB)]
            if len(batches) >= 2 and len(batches[-1]) < GB // 2:
                batches[-2].extend(batches.pop())
            for batch in batches:
                rb0 = batch[0][0]
                RB = sum(g for _, g in batch)
                cm = cm_pool.tile([128, RB * 64], mybir.dt.float16)
                cmoff = 0
                for r0, g in batch:
                    N = g * SW
                    ps = psum_pool.tile(
                        [128, N], mybir.dt.float32, padded_shape=[128, 512]
                    )
                    # fused K=128,M=128 matmuls: both output-parity strips
                    # in one pass of the rhs (lhsT = [A_dy | C_dy]); then the
                    # K=64 third-tap matmuls (B on odd partitions shifted -1
                    # into the even strip, D on even partitions shifted +1
                    # into the odd strip) packed into disjoint array
                    # quadrants via tile_position. The matmuls are chained
                    # with order-only deps so start/stop flags execute
                    # first/last.
                    mms = []

                    def _ac(dy, start, stop):
                        ib = OFF + (r0 + dy) * SW
                        mms.append(nc.tensor.matmul(
                            ps[:, :],
                            wt_sb[:, dy * 128 : dy * 128 + 128],
                            mega[:, ib : ib + N],
                            start=start,
                            stop=stop,
                        ))

                    _ac(0, True, False)
                    for dy in range(3):
                        ib = OFF + (r0 + dy) * SW
                        mms.append(nc.tensor.matmul(
                            ps[0:64, :],
                            wt_sb[64:128, 384 + dy * 64 : 384 + dy * 64 + 64],
                            mega[64:128, ib - 1 : ib - 1 + N],
                            start=False,
                            stop=False,
                            tile_position=(64, 0),
                        ))
                        mms.append(nc.tensor.matmul(
                            ps[64:128, :],
                            wt_sb[0:64, 576 + dy * 64 : 576 + dy * 64 + 64],
                            mega[0:64, ib + 1 : ib + 1 + N],
                            start=False,
                            stop=False,
                            tile_position=(0, 64),
                        ))
                    _ac(1, False, False)
                    _ac(2, False, True)
                    for a, b in zip(mms[1:], mms[:-1]):
                        add_dep_helper(
                            a.ins, b.ins, sync=False, reason="psum group order"
                        )

                    # strip pads + cast into the batch's cm tile
                    ps_v = ps[:].rearrange("p (s c) -> p s c", c=SW)[:, :, 1:65]
                    nc.vector.tensor_copy(
                        out=cm[:, cmoff : cmoff + g * 64].rearrange(
                            "p (s c) -> p s c", c=64
                        ),
                        in_=ps_v,
                    )
                    cmoff += g * 64

                # one batched out-xbar + one batched store for the whole batch
                ot = ot_pool.tile([128, RB * 64], mybir.dt.float16)
                nc.sync.dma_start_transpose(
                    out=ot[:].rearrange("p (g f) -> p g f", f=128),
                    in_=cm[:],
                )
                ydst = y[img, rb0 : rb0 + RB].rearrange(
                    "(cc r) (j two) c -> r j cc two c", r=2, two=2
                ).rearrange("r j cc two c -> (r j) cc (two c)")
                nc.scalar.dma_start(out=ydst, in_=ot[:].rearrange(
                    "p (cc f) -> p cc f", f=128
                ))

    return nc


def make_wt(w_np):
    """Host-side weight prep: w [3,3,64,64] fp32 -> wt [128, 768] bf16."""
    ws = np.sign(w_np).astype(np.float32)
    wt = np.zeros((128, 12 * C), np.float32)
    for dy in range(3):
        # fused [A_dy | C_dy] [128, 128] at cols dy*128:
        #   A (out cols 0:64, even-pixel outputs): rows 0:64 <- w[dy,1],
        #     rows 64:128 <- w[dy,2]
        #   C (out cols 64:128, odd-pixel outputs): rows 0:64 <- w[dy,0],
        #     rows 64:128 <- w[dy,1]
        wt[0:64, dy * 128 : dy * 128 + 64] = ws[dy, 1]
        wt[64:128, dy * 128 : dy * 128 + 64] = ws[dy, 2]
        wt[0:64, dy * 128 + 64 : dy * 128 + 128] = ws[dy, 0]
        wt[64:128, dy * 128 + 64 : dy * 128 + 128] = ws[dy, 1]
        # B: even strip K=64 on odd partitions (w[dy,0])
        wt[64:128, 384 + dy * 64 : 384 + dy * 64 + 64] = ws[dy, 0]
        # D: odd strip K=64 on even partitions (w[dy,2])
        wt[0:64, 576 + dy * 64 : 576 + dy * 64 + 64] = ws[dy, 2]
    return wt.astype(ml_dtypes.bfloat16)


_NC_CACHE = {}


def get_nc():
    if "nc" not in _NC_CACHE:
        _NC_CACHE["nc"] = build_nc()
    return _NC_CACHE["nc"]


def kernel(x, w):
    from concourse.bass_utils import run_bass_kernel_spmd

    x = np.asarray(x, dtype=np.float32)
    w = np.asarray(w, dtype=np.float32)
    assert x.shape == (N_CORES * NIMG, H, W, C) and w.shape == (3, 3, C, C)

    wt = make_wt(w)
    nc = get_nc()
    in_maps = [
        {"x": np.ascontiguousarray(x[c * NIMG : (c + 1) * NIMG]), "wt": wt}
        for c in range(N_CORES)
    ]
    res = run_bass_kernel_spmd(nc, in_maps, list(range(N_CORES)))
    out = np.concatenate(
        [np.asarray(res.results[c]["y"]) for c in range(N_CORES)], axis=0
    )
    return out.astype(np.float32)


# revision 11
# speedup vs baseline: 4.2802x; 1.1353x over previous
"""nn_BinaryConv2D Trainium2 kernel.

out = conv2d(sign(x), sign(w)), 3x3, stride 1, SAME, NHWC/HWIO.
x [64, 128, 128, 64] fp32, w [3, 3, 64, 64] fp32 -> out [64, 128, 128, 64] fp32.

Sharding: data-parallel over batch across 8 NeuronCores (8 images/core);
the tiny weight is sign()ed host-side, packed into per-tap stacks, and
replicated to every core.

Per-core scheme (all shapes hardcoded):
- Input rows are loaded 8 at a time as [128, 512] fp32 SBUF tiles whose
  partition is the pixel-pair index within each 2-row subblock (512B
  contiguous DRAM per partition chunk).
- ACT Sign converts to +-1 bf16.
- DMA xbar transpose turns each 2-row chunk [128 pairs, 128=(2px,ci)] into a
  polyphase stack [128=(even-pixel ci | odd-pixel ci), 128 pairs].
- DVE places phase stacks into a per-image "mega" tile with 66-col slots
  (64 pair columns + shared zero-pad columns) and zero slots for the SAME
  padding rows -1/128.
- Conv = 12 matmuls per PSUM group of g row-slots (N=g*66): for each dy one
  K=128 matmul (two dx taps via the two pixel phases) plus one K=64 matmul
  (third dx tap, shifted one pair column) per output-parity strip; strips
  are packed into PSUM partitions 0:64 / 64:128 (PE column tiling).
- DVE strips pad columns and casts PSUM fp32 -> fp16 (exact: outputs are
  integers in [-576, 576]).
- DMA xbar transpose converts channel-major [co-stacks, pairs] to
  pixel-major [pairs, 2px*co] tiles stored straight into NHWC DRAM (fp16,
  upcast to fp32 on the host — exact).
"""

from contextlib import ExitStack

import numpy as np
import ml_dtypes

import concourse.bass as bass
import concourse.tile as tile
from concourse import mybir
from concourse.vector_clock import ScopedClock, VectorClock
from concourse.tile_rust import add_dep_helper

H = W = 128
C = 64
SW = 66
OFF = 1
MEGA_COLS = OFF + (H + 2) * SW + 1  # 8582
N_CORES = 8
NIMG = 8  # images per core


# ---------------------------------------------------------------------------
# Workaround for this container's walrus: CTRL instructions support only ONE
# sync-wait slot, but Tile's tail drain attaches one wait per live proc.
# Split the waits across single-wait NoOps on the SP engine (in-order), then
# drain waitless.
def _drain_and_barrier_split(self, tick_clock, wait_clock):
    nc = self.nc
    vc = tick_clock.global_clock
    n = len(vc)
    for i in range(n):
        if vc[i] > 0:
            sub = VectorClock([0] * n)
            sub.require_at_least(i, vc[i])
            nop = nc.sync.nop(nofuse=True)
            wait_clock.add_sem_waits(nop.ins, ScopedClock({None: sub}))
    nc.sync.drain()
    nc.all_engine_barrier()
    assert self.sems is not None
    popped = nc._tile_sem_poison_stack.pop()
    assert popped is self._sem_poison
    nc.clear_and_free_semaphores(list(self.sems.allocated().values()))
    nc.all_engine_barrier()


tile.TileContext._drain_and_barrier = _drain_and_barrier_split


# The same walrus limit applies to every instruction: at most one sync wait.
# Tile freely emits multi-wait instructions, so rewrite the BIR JSON right
# before compilation: hoist all but the last wait of each instruction onto
# fresh same-engine NoOps inserted immediately before it (engines execute
# their instruction stream in order, so the waits still gate the original
# instruction).
def _split_multi_waits_json(bir_bytes):
    import json as _json

    bir = _json.loads(bir_bytes)
    n = 0
    for fn in bir.get("functions", []):
        for blk in fn.get("blocks", []):
            insts = blk.get("instructions", [])
            out = []
            for ins in insts:
                si = ins.get("sync_info")
                if si:
                    waits = si.get("on_wait") or []
                    if len(waits) > 1:
                        for wv in waits[:-1]:
                            n += 1
                            out.append(
                                {
                                    "debug": ins.get("debug", 0),
                                    "engine": ins["engine"],
                                    "ins": [],
                                    "outs": [],
                                    "name": f"I-wsplit-{n}",
                                    "opcode": "NoOp",
                                    "sync_info": {
                                        "on_update": [],
                                        "on_wait": [wv],
                                    },
                                }
                            )
                        si["on_wait"] = [waits[-1]]
                out.append(ins)
            blk["instructions"] = out
    return _json.dumps(bir).encode()


def _install_compile_hook():
    from concourse import bass_utils as _bu
    from concourse import bass2jax as _b2j

    if getattr(_bu, "_orig_compile_bir_kernel", None) is None:
        _bu._orig_compile_bir_kernel = _bu.compile_bir_kernel

        def _patched(bir_json, tmpdir, neff_name="file.neff"):
            return _bu._orig_compile_bir_kernel(
                _split_multi_waits_json(bir_json), tmpdir, neff_name=neff_name
            )

        _bu.compile_bir_kernel = _patched
        _b2j.compile_bir_kernel = _patched


_install_compile_hook()
# ---------------------------------------------------------------------------


def build_nc(nimg=NIMG, gsize=6, mega_bufs=2, psum_bufs=4, io_bufs=3,
             cm_bufs=3, ot_bufs=6, rc=32, gb=4):
    nc = bass.Bass()
    x = nc.dram_tensor("x", [nimg, H, W, C], mybir.dt.float32, kind="ExternalInput")
    wt = nc.dram_tensor("wt", [128, 12 * C], mybir.dt.bfloat16, kind="ExternalInput")
    y = nc.dram_tensor("y", [nimg, H, W, C], mybir.dt.float16, kind="ExternalOutput")

    with tile.TileContext(nc) as tc, ExitStack() as ctx:
        wpool = ctx.enter_context(tc.tile_pool(name="wpool", bufs=1))
        mega_pool = ctx.enter_context(tc.tile_pool(name="mega", bufs=mega_bufs))
        in_pool = ctx.enter_context(tc.tile_pool(name="inp", bufs=io_bufs))
        sg_pool = ctx.enter_context(tc.tile_pool(name="sg", bufs=io_bufs))
        tmp_pool = ctx.enter_context(tc.tile_pool(name="tmp", bufs=io_bufs))
        psum_pool = ctx.enter_context(
            tc.tile_pool(name="ps", bufs=psum_bufs, space="PSUM")
        )
        cm_pool = ctx.enter_context(tc.tile_pool(name="cm", bufs=cm_bufs))
        ot_pool = ctx.enter_context(tc.tile_pool(name="ot", bufs=ot_bufs))

        wt_sb = wpool.tile([128, 12 * C], mybir.dt.bfloat16)
        nc.sync.dma_start(out=wt_sb[:], in_=wt[:])

        groups = []
        r0 = 0
        while r0 < H:
            g = min(gsize, H - r0)
            groups.append((r0, g))
            r0 += g

        for img in range(nimg):
            mega = mega_pool.tile([128, MEGA_COLS], mybir.dt.bfloat16)
            # zero-pad columns: {66m+1} (left pads) and {66m+66} (right pads)
            # covered as pairs {66m+66, 66m+67} plus edge cols.
            nc.gpsimd.memset(mega[:, 0:2], 0.0)
            pads = mega[:, SW : SW + 129 * SW].rearrange(
                "p (s c) -> p s c", c=SW
            )[:, :, 0:2]
            nc.gpsimd.memset(pads, 0.0)
            nc.gpsimd.memset(mega[:, MEGA_COLS - 2 : MEGA_COLS], 0.0)
            # zero slots for SAME-pad rows -1 and H
            nc.gpsimd.memset(mega[:, 2 : 2 + 64], 0.0)
            b128 = OFF + (H + 1) * SW + 1
            nc.gpsimd.memset(mega[:, b128 : b128 + 64], 0.0)

            # input: chunks of rc rows
            RC = rc  # rows per chunk
            FC = RC * 64  # free elems per chunk (= 16 blocks of 128)
            for t in range(H // RC):
                xt = in_pool.tile([128, FC], mybir.dt.float32)
                src = x[img, RC * t : RC * t + RC]  # [RC, 128, 64]
                src_v = src.rearrange(
                    "(u r) (j two) c -> r j u two c", r=2, two=2
                ).rearrange("r j u two c -> (r j) u (two c)")
                nc.sync.dma_start(
                    out=xt[:].rearrange("p (u f) -> p u f", f=128), in_=src_v
                )

                sg = sg_pool.tile([128, FC], mybir.dt.bfloat16)
                nc.scalar.activation(
                    sg[:], xt[:], mybir.ActivationFunctionType.Sign
                )

                # one batched xbar: G independent 128x128 transposes
                tmp = tmp_pool.tile([128, FC], mybir.dt.bfloat16)
                nc.sync.dma_start_transpose(
                    out=tmp[:].rearrange("p (g f) -> p g f", f=128),
                    in_=sg[:],
                )

                b0 = OFF + (RC * t + 1) * SW + 1
                dst = mega[:, b0 : b0 + RC * SW].rearrange(
                    "p (s c) -> p s c", c=SW
                )[:, :, 0:64]
                nc.vector.tensor_copy(
                    out=dst, in_=tmp[:].rearrange("p (s c) -> p s c", c=64)
                )

            # matmul groups, outputs batched across GB groups per cm tile
            GB = gb
            batches = [groups[i : i + GB] for i in range(0, len(groups), GB)]
            if len(batches) >= 2 and len(batches[-1]) < GB // 2:
                batches[-2].extend(batches.pop())
            for batch in batches:
                rb0 = batch[0][0]
                RB = sum(g for _, g in batch)
                cm = cm_pool.tile([128, RB * 64], mybir.dt.float16)
                cmoff = 0
                for r0, g in batch:
                    N = g * SW
                    ps = psum_pool.tile(
                        [128, N], mybir.dt.float32, padded_shape=[128, 512]
                    )
                    # fused K=128,M=128 matmuls: both output-parity strips
                    # in one pass of the rhs (lhsT = [A_dy | C_dy]); then the
                    # K=64 third-tap matmuls (B on odd partitions shifted -1
                    # into the even strip, D on even partitions shifted +1
                    # into the odd strip) packed into disjoint array
                    # quadrants via tile_position. The matmuls are chained
                    # with order-only deps so start/stop flags execute
                    # first/last.
                    mms = []

                    def _ac(dy, start, stop):
                        ib = OFF + (r0 + dy) * SW
                        mms.append(nc.tensor.matmul(
                            ps[:, :],
                            wt_sb[:, dy * 128 : dy * 128 + 128],
                            mega[:, ib : ib + N],
                            start=start,
                            stop=stop,
                        ))

                    _ac(0, True, False)
                    for dy in range(3):
                        ib = OFF + (r0 + dy) * SW
                        mms.append(nc.tensor.matmul(
                            ps[0:64, :],
                            wt_sb[64:128, 384 + dy * 64 : 384 + dy * 64 + 64],
                            mega[64:128, ib - 1 : ib - 1 + N],
                            start=False,
                            stop=False,
                            tile_position=(64, 0),
                        ))
                        mms.append(nc.tensor.matmul(
                            ps[64:128, :],
                            wt_sb[0:64, 576 + dy * 64 : 576 + dy * 64 + 64],
                            mega[0:64, ib + 1 : ib + 1 + N],
                            start=False,
                            stop=False,
                            tile_position=(0, 64),
                        ))
                    _ac(1, False, False)
                    _ac(2, False, True)
                    for a, b in zip(mms[1:], mms[:-1]):
                        add_dep_helper(
                            a.ins, b.ins, sync=False, reason="psum group order"
                        )

                    # strip pads + cast into the batch's cm tile
                    ps_v = ps[:].rearrange("p (s c) -> p s c", c=SW)[:, :, 1:65]
                    nc.vector.tensor_copy(
                        out=cm[:, cmoff : cmoff + g * 64].rearrange(
                            "p (s c) -> p s c", c=64
                        ),
                        in_=ps_v,
                    )
                    cmoff += g * 64

                # one batched out-xbar + one batched store for the whole batch
                ot = ot_pool.tile([128, RB * 64], mybir.dt.float16)
                nc.sync.dma_start_transpose(
                    out=ot[:].rearrange("p (g f) -> p g f", f=128),
                    in_=cm[:],
                )
                ydst = y[img, rb0 : rb0 + RB].rearrange(
                    "(cc r) (j two) c -> r j cc two c", r=2, two=2
                ).rearrange("r j cc two c -> (r j) cc (two c)")
                nc.scalar.dma_start(out=ydst, in_=ot[:].rearrange(
                    "p (cc f) -> p cc f", f=128
                ))

    return nc


def make_wt(w_np):
    """Host-side weight prep: w [3,3,64,64] fp32 -> wt [128, 768] bf16."""
    ws = np.sign(w_np).astype(np.float32)
    wt = np.zeros((128, 12 * C), np.float32)
    for dy in range(3):
        # fused [A_dy | C_dy] [128, 128] at cols dy*128:
        #   A (out cols 0:64, even-pixel outputs): rows 0:64 <- w[dy,1],
        #     rows 64:128 <- w[dy,2]
        #   C (out cols 64:128, odd-pixel outputs): rows 0:64 <- w[dy,0],
        #     rows 64:128 <- w[dy,1]
        wt[0:64, dy * 128 : dy * 128 + 64] = ws[dy, 1]
        wt[64:128, dy * 128 : dy * 128 + 64] = ws[dy, 2]
        wt[0:64, dy * 128 + 64 : dy * 128 + 128] = ws[dy, 0]
        wt[64:128, dy * 128 + 64 : dy * 128 + 128] = ws[dy, 1]
        # B: even strip K=64 on odd partitions (w[dy,0])
        wt[64:128, 384 + dy * 64 : 384 + dy * 64 + 64] = ws[dy, 0]
        # D: odd strip K=64 on even partitions (w[dy,2])
        wt[0:64, 576 + dy * 64 : 576 + dy * 64 + 64] = ws[dy, 2]
    return wt.astype(ml_dtypes.bfloat16)


_NC_CACHE = {}


def get_nc():
    if "nc" not in _NC_CACHE:
        _NC_CACHE["nc"] = build_nc()
    return _NC_CACHE["nc"]


def kernel(x, w):
    from concourse.bass_utils import run_bass_kernel_spmd

    x = np.asarray(x, dtype=np.float32)
    w = np.asarray(w, dtype=np.float32)
    assert x.shape == (N_CORES * NIMG, H, W, C) and w.shape == (3, 3, C, C)

    wt = make_wt(w)
    nc = get_nc()
    in_maps = [
        {"x": np.ascontiguousarray(x[c * NIMG : (c + 1) * NIMG]), "wt": wt}
        for c in range(N_CORES)
    ]
    res = run_bass_kernel_spmd(nc, in_maps, list(range(N_CORES)))
    out = np.concatenate(
        [np.asarray(res.results[c]["y"]) for c in range(N_CORES)], axis=0
    )
    return out.astype(np.float32)


# revision 17
# speedup vs baseline: 5.2847x; 1.2347x over previous
"""nn_BinaryConv2D Trainium2 kernel.

out = conv2d(sign(x), sign(w)), 3x3, stride 1, SAME, NHWC/HWIO.
x [64, 128, 128, 64] fp32, w [3, 3, 64, 64] fp32 -> out [64, 128, 128, 64] fp32.

Sharding: data-parallel over batch across 8 NeuronCores (8 images/core);
the tiny weight is sign()ed host-side, packed into per-tap stacks, and
replicated to every core.

Per-core scheme (all shapes hardcoded):
- Input rows are loaded 8 at a time as [128, 512] fp32 SBUF tiles whose
  partition is the pixel-pair index within each 2-row subblock (512B
  contiguous DRAM per partition chunk).
- ACT Sign converts to +-1 bf16.
- DMA xbar transpose turns each 2-row chunk [128 pairs, 128=(2px,ci)] into a
  polyphase stack [128=(even-pixel ci | odd-pixel ci), 128 pairs].
- DVE places phase stacks into a per-image "mega" tile with 66-col slots
  (64 pair columns + shared zero-pad columns) and zero slots for the SAME
  padding rows -1/128.
- Conv = 12 matmuls per PSUM group of g row-slots (N=g*66): for each dy one
  K=128 matmul (two dx taps via the two pixel phases) plus one K=64 matmul
  (third dx tap, shifted one pair column) per output-parity strip; strips
  are packed into PSUM partitions 0:64 / 64:128 (PE column tiling).
- DVE strips pad columns and casts PSUM fp32 -> fp16 (exact: outputs are
  integers in [-576, 576]).
- DMA xbar transpose converts channel-major [co-stacks, pairs] to
  pixel-major [pairs, 2px*co] tiles stored straight into NHWC DRAM (fp16,
  upcast to fp32 on the host — exact).
"""

from contextlib import ExitStack

import numpy as np
import ml_dtypes

import concourse.bass as bass
import concourse.tile as tile
from concourse import mybir
from concourse.vector_clock import ScopedClock, VectorClock
from concourse.tile_rust import add_dep_helper

H = W = 128
C = 64
SW = 66
OFF = 1
MEGA_COLS = OFF + (H + 2) * SW + 1  # 8582
N_CORES = 8
NIMG = 8  # images per core


# ---------------------------------------------------------------------------
# Workaround for this container's walrus: CTRL instructions support only ONE
# sync-wait slot, but Tile's tail drain attaches one wait per live proc.
# Split the waits across single-wait NoOps on the SP engine (in-order), then
# drain waitless.
def _drain_and_barrier_split(self, tick_clock, wait_clock):
    nc = self.nc
    vc = tick_clock.global_clock
    n = len(vc)
    for i in range(n):
        if vc[i] > 0:
            sub = VectorClock([0] * n)
            sub.require_at_least(i, vc[i])
            nop = nc.sync.nop(nofuse=True)
            wait_clock.add_sem_waits(nop.ins, ScopedClock({None: sub}))
    nc.sync.drain()
    nc.all_engine_barrier()
    assert self.sems is not None
    popped = nc._tile_sem_poison_stack.pop()
    assert popped is self._sem_poison
    nc.clear_and_free_semaphores(list(self.sems.allocated().values()))
    nc.all_engine_barrier()


tile.TileContext._drain_and_barrier = _drain_and_barrier_split


# The same walrus limit applies to every instruction: at most one sync wait.
# Tile freely emits multi-wait instructions, so rewrite the BIR JSON right
# before compilation: hoist all but the last wait of each instruction onto
# fresh same-engine NoOps inserted immediately before it (engines execute
# their instruction stream in order, so the waits still gate the original
# instruction).
def _split_multi_waits_json(bir_bytes):
    import json as _json

    bir = _json.loads(bir_bytes)
    n = 0
    for fn in bir.get("functions", []):
        for blk in fn.get("blocks", []):
            insts = blk.get("instructions", [])
            out = []
            for ins in insts:
                si = ins.get("sync_info")
                if si:
                    waits = si.get("on_wait") or []
                    if len(waits) > 1:
                        for wv in waits[:-1]:
                            n += 1
                            out.append(
                                {
                                    "debug": ins.get("debug", 0),
                                    "engine": ins["engine"],
                                    "ins": [],
                                    "outs": [],
                                    "name": f"I-wsplit-{n}",
                                    "opcode": "NoOp",
                                    "sync_info": {
                                        "on_update": [],
                                        "on_wait": [wv],
                                    },
                                }
                            )
                        si["on_wait"] = [waits[-1]]
                out.append(ins)
            blk["instructions"] = out
    return _json.dumps(bir).encode()


def _install_compile_hook():
    from concourse import bass_utils as _bu
    from concourse import bass2jax as _b2j

    if getattr(_bu, "_orig_compile_bir_kernel", None) is None:
        _bu._orig_compile_bir_kernel = _bu.compile_bir_kernel

        def _patched(bir_json, tmpdir, neff_name="file.neff"):
            return _bu._orig_compile_bir_kernel(
                _split_multi_waits_json(bir_json), tmpdir, neff_name=neff_name
            )

        _bu.compile_bir_kernel = _patched
        _b2j.compile_bir_kernel = _patched


_install_compile_hook()
# ---------------------------------------------------------------------------


def build_nc(nimg=NIMG, gsize=6, mega_bufs=2, psum_bufs=4, io_bufs=5,
             cm_bufs=3, ot_bufs=6, rc=32, gb=11, use_dr=False,
             pe_inx=True):
    nc = bass.Bass()
    x = nc.dram_tensor("x", [nimg, H, W, C], mybir.dt.float32, kind="ExternalInput")
    wdt = mybir.dt.float8e4 if use_dr else mybir.dt.bfloat16
    mdt = mybir.dt.float8e4 if use_dr else mybir.dt.bfloat16
    wcols = 18 * C if use_dr else 12 * C
    wt = nc.dram_tensor("wt", [128, wcols], wdt, kind="ExternalInput")
    ident = nc.dram_tensor(
        "ident", [128, 128], mybir.dt.float32, kind="ExternalInput"
    )
    y = nc.dram_tensor("y", [nimg, H, W, C], mybir.dt.float16, kind="ExternalOutput")

    with tile.TileContext(nc) as tc, ExitStack() as ctx:
        wpool = ctx.enter_context(tc.tile_pool(name="wpool", bufs=1))
        mega_pool = ctx.enter_context(tc.tile_pool(name="mega", bufs=mega_bufs))
        in_pool = ctx.enter_context(tc.tile_pool(name="inp", bufs=io_bufs))
        sg_pool = ctx.enter_context(tc.tile_pool(name="sg", bufs=io_bufs))
        tmp_pool = ctx.enter_context(tc.tile_pool(name="tmp", bufs=io_bufs))
        psum_pool = ctx.enter_context(
            tc.tile_pool(name="ps", bufs=psum_bufs, space="PSUM")
        )
        pin_pool = ctx.enter_context(
            tc.tile_pool(name="pin", bufs=3, space="PSUM")
        )
        cm_pool = ctx.enter_context(tc.tile_pool(name="cm", bufs=cm_bufs))
        ot_pool = ctx.enter_context(tc.tile_pool(name="ot", bufs=ot_bufs))

        wt_sb = wpool.tile([128, wcols], wdt)
        nc.sync.dma_start(out=wt_sb[:], in_=wt[:])
        ident_sb = wpool.tile([128, 128], mybir.dt.float32)
        if pe_inx:
            nc.sync.dma_start(out=ident_sb[:], in_=ident[:])

        groups = []
        r0 = 0
        while r0 < H:
            g = min(gsize, H - r0)
            groups.append((r0, g))
            r0 += g

        def input_stage(img):
            mega = mega_pool.tile([128, MEGA_COLS], mdt)
            # zero-pad columns: {66m+1} (left pads) and {66m+66} (right pads)
            # covered as pairs {66m+66, 66m+67} plus edge cols.
            nc.gpsimd.memset(mega[:, 0:2], 0.0)
            pads = mega[:, SW : SW + 129 * SW].rearrange(
                "p (s c) -> p s c", c=SW
            )[:, :, 0:2]
            nc.gpsimd.memset(pads, 0.0)
            nc.gpsimd.memset(mega[:, MEGA_COLS - 2 : MEGA_COLS], 0.0)
            # zero slots for SAME-pad rows -1 and H
            nc.gpsimd.memset(mega[:, 2 : 2 + 64], 0.0)
            b128 = OFF + (H + 1) * SW + 1
            nc.gpsimd.memset(mega[:, b128 : b128 + 64], 0.0)

            # input: chunks of rc rows
            RC = rc  # rows per chunk
            FC = RC * 64  # free elems per chunk (= 16 blocks of 128)
            for t in range(H // RC):
                xt = in_pool.tile([128, FC], mybir.dt.float32)
                src = x[img, RC * t : RC * t + RC]  # [RC, 128, 64]
                src_v = src.rearrange(
                    "(u r) (j two) c -> r j u two c", r=2, two=2
                ).rearrange("r j u two c -> (r j) u (two c)")
                nc.sync.dma_start(
                    out=xt[:].rearrange("p (u f) -> p u f", f=128), in_=src_v
                )

                if pe_inx:
                    # PE transposes of raw fp32 into PSUM (4 banks of 4
                    # blocks each), then fused Sign+place on ACT: PSUM
                    # [128=(2px,ci), pairs] -> mega slots (fp8, +-1).
                    for q in range(FC // 512):
                        pin = pin_pool.tile([128, 512], mybir.dt.float32)
                        for u in range(4):
                            nc.tensor.transpose(
                                pin[:, 128 * u : 128 * (u + 1)],
                                xt[:, 512 * q + 128 * u : 512 * q + 128 * (u + 1)],
                                ident_sb[:],
                            )
                        rr0 = RC * t + 8 * q
                        b0 = OFF + (rr0 + 1) * SW + 1
                        dst = mega[:, b0 : b0 + 8 * SW].rearrange(
                            "p (s c) -> p s c", c=SW
                        )[:, :, 0:64]
                        nc.scalar.activation(
                            dst,
                            pin[:].rearrange("p (s c) -> p s c", c=64),
                            mybir.ActivationFunctionType.Sign,
                        )
                else:
                    sg = sg_pool.tile([128, FC], mybir.dt.bfloat16)
                    nc.scalar.activation(
                        sg[:], xt[:], mybir.ActivationFunctionType.Sign
                    )

                    # one batched xbar: G independent 128x128 transposes
                    tmp = tmp_pool.tile([128, FC], mybir.dt.bfloat16)
                    nc.sync.dma_start_transpose(
                        out=tmp[:].rearrange("p (g f) -> p g f", f=128),
                        in_=sg[:],
                    )

                    b0 = OFF + (RC * t + 1) * SW + 1
                    dst = mega[:, b0 : b0 + RC * SW].rearrange(
                        "p (s c) -> p s c", c=SW
                    )[:, :, 0:64]
                    nc.vector.tensor_copy(
                        out=dst, in_=tmp[:].rearrange("p (s c) -> p s c", c=64)
                    )

            return mega

        def compute_stage(img, mega):
            # matmul groups, outputs batched across GB groups per cm tile
            GB = gb
            batches = [groups[i : i + GB] for i in range(0, len(groups), GB)]
            if len(batches) >= 2 and len(batches[-1]) < GB // 2:
                batches[-2].extend(batches.pop())
            for batch in batches:
                rb0 = batch[0][0]
                RB = sum(g for _, g in batch)
                cm = cm_pool.tile([128, RB * 64], mybir.dt.float16)
                cmoff = 0
                for r0, g in batch:
                    N = g * SW
                    ps = psum_pool.tile(
                        [128, N], mybir.dt.float32, padded_shape=[128, 512]
                    )
                    # fused K=128,M=128 matmuls: both output-parity strips
                    # in one pass of the rhs (lhsT = [A_dy | C_dy]); then the
                    # K=64 third-tap matmuls (B on odd partitions shifted -1
                    # into the even strip, D on even partitions shifted +1
                    # into the odd strip) packed into disjoint array
                    # quadrants via tile_position. The matmuls are chained
                    # with order-only deps so start/stop flags execute
                    # first/last.
                    mms = []
                    if use_dr:
                        # fp8 DoubleRow, full-width M=128: the k-pair window
                        # (j-1, j) covers all 3 even-parity dx taps and 2 of
                        # the odd-parity taps in one matmul; a plain fp8
                        # matmul at window j+1 adds the remaining odd tap
                        # (its even-output columns are zero-weighted).
                        for dy in range(3):
                            ib = OFF + (r0 + dy) * SW
                            sl = mega[:, ib - 1 : ib - 1 + N]
                            rhs = bass.AP(
                                sl.tensor,
                                sl.offset,
                                [list(sl.ap[0]), [1, 2], [1, N]],
                            )
                            wsl = wt_sb[
                                :, dy * 256 : dy * 256 + 256
                            ].rearrange("p (k m) -> p k m", k=2)
                            mms.append(nc.tensor.matmul(
                                ps[:, :],
                                wsl,
                                rhs,
                                start=(dy == 0),
                                stop=False,
                                perf_mode=mybir.MatmulPerfMode.DoubleRow,
                            ))
                            mms.append(nc.tensor.matmul(
                                ps[:, :],
                                wt_sb[:, 768 + dy * 128 : 768 + dy * 128 + 128],
                                mega[:, ib + 1 : ib + 1 + N],
                                start=False,
                                stop=(dy == 2),
                            ))
                    else:
                        def _ac(dy, start, stop):
                            ib = OFF + (r0 + dy) * SW
                            mms.append(nc.tensor.matmul(
                                ps[:, :],
                                wt_sb[:, dy * 128 : dy * 128 + 128],
                                mega[:, ib : ib + N],
                                start=start,
                                stop=stop,
                            ))

                        _ac(0, True, False)
                        for dy in range(3):
                            ib = OFF + (r0 + dy) * SW
                            mms.append(nc.tensor.matmul(
                                ps[0:64, :],
                                wt_sb[64:128, 384 + dy * 64 : 384 + dy * 64 + 64],
                                mega[64:128, ib - 1 : ib - 1 + N],
                                start=False,
                                stop=False,
                                tile_position=(64, 0),
                            ))
                            mms.append(nc.tensor.matmul(
                                ps[64:128, :],
                                wt_sb[0:64, 576 + dy * 64 : 576 + dy * 64 + 64],
                                mega[0:64, ib + 1 : ib + 1 + N],
                                start=False,
                                stop=False,
                                tile_position=(0, 64),
                            ))
                        _ac(1, False, False)
                        _ac(2, False, True)
                    for a, b in zip(mms[1:], mms[:-1]):
                        add_dep_helper(
                            a.ins, b.ins, sync=False, reason="psum group order"
                        )

                    # strip pads + cast into the batch's cm tile
                    ps_v = ps[:].rearrange("p (s c) -> p s c", c=SW)[:, :, 1:65]
                    nc.vector.tensor_copy(
                        out=cm[:, cmoff : cmoff + g * 64].rearrange(
                            "p (s c) -> p s c", c=64
                        ),
                        in_=ps_v,
                    )
                    cmoff += g * 64

                # one batched out-xbar + one batched store for the whole batch
                ot = ot_pool.tile([128, RB * 64], mybir.dt.float16)
                nc.sync.dma_start_transpose(
                    out=ot[:].rearrange("p (g f) -> p g f", f=128),
                    in_=cm[:],
                )
                ydst = y[img, rb0 : rb0 + RB].rearrange(
                    "(cc r) (j two) c -> r j cc two c", r=2, two=2
                ).rearrange("r j cc two c -> (r j) cc (two c)")
                nc.scalar.dma_start(out=ydst, in_=ot[:].rearrange(
                    "p (cc f) -> p cc f", f=128
                ))

        # software-pipelined emission: input stage of image i+1 is emitted
        # before compute/output of image i so the in-order DMA sequencer
        # doesn't head-of-line block next image's loads behind this image's
        # output transposes.
        megas = {}
        for img in range(nimg + 1):
            if img < nimg:
                megas[img] = input_stage(img)
            if img >= 1:
                compute_stage(img - 1, megas.pop(img - 1))

    return nc


def make_wt_dr(w_np):
    """DoubleRow weight prep: w [3,3,64,64] fp32 -> wt [128, 1152] fp8e4.

    Per dy: W_dy [128, 2, 128] at cols dy*256 (DoubleRow, rhs window j-1/j)
    and V_dy [128, 128] at cols 768+dy*128 (plain matmul, rhs window j+1).
    Output columns m<64 = even-pixel outputs, m>=64 = odd-pixel outputs.
    """
    ws = np.sign(w_np).astype(np.float32)
    wt = np.zeros((128, 18 * C), np.float32)
    for dy in range(3):
        Wd = np.zeros((128, 2, 128), np.float32)
        Wd[64:128, 0, 0:64] = ws[dy, 0]
        Wd[0:64, 1, 0:64] = ws[dy, 1]
        Wd[64:128, 1, 0:64] = ws[dy, 2]
        Wd[0:64, 1, 64:128] = ws[dy, 0]
        Wd[64:128, 1, 64:128] = ws[dy, 1]
        wt[:, dy * 256 : dy * 256 + 256] = Wd.reshape(128, 256)
        wt[0:64, 768 + dy * 128 + 64 : 768 + dy * 128 + 128] = ws[dy, 2]
    fp8 = mybir.dt.np(mybir.dt.float8e4)
    return wt.astype(fp8)


def make_wt(w_np):
    """Host-side weight prep: w [3,3,64,64] fp32 -> wt [128, 768] bf16."""
    ws = np.sign(w_np).astype(np.float32)
    wt = np.zeros((128, 12 * C), np.float32)
    for dy in range(3):
        # fused [A_dy | C_dy] [128, 128] at cols dy*128:
        #   A (out cols 0:64, even-pixel outputs): rows 0:64 <- w[dy,1],
        #     rows 64:128 <- w[dy,2]
        #   C (out cols 64:128, odd-pixel outputs): rows 0:64 <- w[dy,0],
        #     rows 64:128 <- w[dy,1]
        wt[0:64, dy * 128 : dy * 128 + 64] = ws[dy, 1]
        wt[64:128, dy * 128 : dy * 128 + 64] = ws[dy, 2]
        wt[0:64, dy * 128 + 64 : dy * 128 + 128] = ws[dy, 0]
        wt[64:128, dy * 128 + 64 : dy * 128 + 128] = ws[dy, 1]
        # B: even strip K=64 on odd partitions (w[dy,0])
        wt[64:128, 384 + dy * 64 : 384 + dy * 64 + 64] = ws[dy, 0]
        # D: odd strip K=64 on even partitions (w[dy,2])
        wt[0:64, 576 + dy * 64 : 576 + dy * 64 + 64] = ws[dy, 2]
    return wt.astype(ml_dtypes.bfloat16)


_NC_CACHE = {}


def get_nc():
    if "nc" not in _NC_CACHE:
        _NC_CACHE["nc"] = build_nc()
    return _NC_CACHE["nc"]


def kernel(x, w):
    from concourse.bass_utils import run_bass_kernel_spmd

    x = np.asarray(x, dtype=np.float32)
    w = np.asarray(w, dtype=np.float32)
    assert x.shape == (N_CORES * NIMG, H, W, C) and w.shape == (3, 3, C, C)

    wt = make_wt(w)
    nc = get_nc()
    ident = np.eye(128, dtype=np.float32)
    in_maps = [
        {
            "x": np.ascontiguousarray(x[c * NIMG : (c + 1) * NIMG]),
            "wt": wt,
            "ident": ident,
        }
        for c in range(N_CORES)
    ]
    res = run_bass_kernel_spmd(nc, in_maps, list(range(N_CORES)))
    out = np.concatenate(
        [np.asarray(res.results[c]["y"]) for c in range(N_CORES)], axis=0
    )
    return out.astype(np.float32)
